# revision 19
# baseline (speedup 1.0000x reference)
"""Trainium2 Bass kernel for nn_MultiHeadAttention (B=2, C=1024, H=16, S=2048).

Sharding: 8 cores = 2 batches x 4 head-groups (4 heads per core).

v3 design notes (vs baseline):
  - All SBUF allocated up-front except Vin (scoped; its space is reused for
    the attention-phase pt/i16/ob tiles long after vT proj finished), and
    the mask is DMA'd right behind Q/K so the attention phase never stalls
    on SBUF frees.  The baseline's proj->attention PE idle gap tripped the
    clock-gate to K=4/8 which then sticks for ~55 HAM windows (~190us at
    half clock) -- the single biggest cost.
  - PSUM handoff: in the attention scope, cx is allocated BELOW sc so that
    sc aliases only the early-freed pv banks + virgin space; the first
    score matmul then issues back-to-back after the last proj matmul.
  - Everything bf16 (fp8 on any operand feeding attention fails the 2e-2
    gate: attention weight concentrates on few keys, so quantization noise
    does not average out).  q_sb/k_sb bf16 (baseline fp32r cost +130ns/MM
    and +140ns/LDW on score matmuls); 1/sqrt(cph) folded into the exp.
  - V/Q/K DMA'd in 4 S-chunks each, proj emitted per chunk, so the PE
    rides just behind the DMA stream with no idle gap anywhere.
  - Scores computed TRANSPOSED (scoresT[j,i] = k.q) so softmax
    normalization rides the ctx matmul via a ones-column in vT.
  - exp on ACT for most tiles; ~1/6 of tiles use a Schraudolph fast-exp on
    DVE (i16 = round(s*A+B) bitcast to bf16) to balance ACT vs DVE load.
    Mask applied multiplicatively after exp (bf16 2x mode on DVE).
  - Output bf16; host divides by the denominator row and transposes.
"""

import numpy as np
import ml_dtypes

import concourse.bass as bass
import concourse.mybir as mybir
import concourse.tile as tile
from concourse import bacc
from concourse.bass_utils import run_bass_kernel_spmd

B = 2
C = 1024
HEADS = 16
CPH = 64
S = 2048
N_CORES = 8
HPC = 4  # heads per core
CPC = HPC * CPH  # channels per core = 256
HW = 65  # per-head slot width in vT (64 d + 1 ones)
VW = HPC * HW  # 260

BF = mybir.dt.bfloat16
F32 = mybir.dt.float32
I16 = mybir.dt.int16
EXP = mybir.ActivationFunctionType.Exp
MULT = mybir.AluOpType.mult
ADD = mybir.AluOpType.add

NBF = ml_dtypes.bfloat16

ESCALE = 1.0 / 8.0  # 1/sqrt(cph), applied inside the exp
# Schraudolph bf16 fast-exp: exp(x*ESCALE) ~= bitcast_bf16(i16(x*SA + SB))
SA = (128.0 / np.log(2.0)) * ESCALE
SB = 128.0 * (127.0 - 0.0577)  # mean-centering constant


def _schraudolph(j, hh):
    return hh == 1 and j % 3 == 2  # 5 of 32 tiles per (p, hf) group


_NC_CACHE = {}


def build_nc():
    nc = bacc.Bacc("TRN2", target_bir_lowering=False)

    Qd = nc.declare_dram_parameter("Qin", [C, S], BF, isOutput=False)
    Kd = nc.declare_dram_parameter("Kin", [C, S], BF, isOutput=False)
    Vd = nc.declare_dram_parameter("Vin", [C, S], BF, isOutput=False)
    WqTd = nc.declare_dram_parameter("WqT", [C, CPC], BF, isOutput=False)
    WkTd = nc.declare_dram_parameter("WkT", [C, CPC], BF, isOutput=False)
    WvTd = nc.declare_dram_parameter("WvT", [C, VW], BF, isOutput=False)
    bqkd = nc.declare_dram_parameter("bqk", [128, 4], F32, isOutput=False)
    bvbd = nc.declare_dram_parameter("bvb", [128, VW], F32, isOutput=False)
    Md = nc.declare_dram_parameter("maskT", [S, S], BF, isOutput=False)
    Od = nc.declare_dram_parameter("out", [VW, S], BF, isOutput=True)

    with tile.TileContext(nc) as tc:
        with (
            tc.tile_pool(name="w", bufs=1) as wp,
            tc.tile_pool(name="io", bufs=1) as io,
            tc.tile_pool(name="qksb", bufs=1) as qkp,
            tc.tile_pool(name="msk", bufs=1) as mkp,
        ):
            # --- persistent SBUF ---
            WvT = wp.tile([128, 8, VW], BF, tag="wv")
            WqT = wp.tile([128, 8, CPC], BF, tag="wq")
            WkT = wp.tile([128, 8, CPC], BF, tag="wk")
            bqk = wp.tile([128, 4], F32, tag="bqk")
            bvb = wp.tile([128, VW], F32, tag="bvb")
            Qin = io.tile([128, 8, S], BF, tag="qi")
            Kin = io.tile([128, 8, S], BF, tag="ki")
            q_sb = qkp.tile([128, 2, S], BF, tag="q")  # pair-major, head rows 0:64/64:128
            k_sb = qkp.tile([128, 2, S], BF, tag="k")
            vT = qkp.tile([128, 16, VW], BF, tag="vt")  # s_tile-major
            maskT = mkp.tile([128, 16, S], BF, tag="m")

            with tc.tile_pool(name="vio", bufs=1) as vio:
                Vin = vio.tile([128, 8, S], BF, tag="vi")
                escratch = vio.tile([128, 4], BF, tag="esc")

                # --- DMA, in need-order (the shared HW queue drains roughly
                # in posting order at full aggregate bandwidth); few, large
                # posts -- each posting costs ~750ns on the sync engine ---
                nc.sync.dma_start(
                    WvT[:], WvTd[:].rearrange("(t p) n -> p t n", p=128)
                )
                nc.sync.dma_start(bvb[:], bvbd[:])
                for n4 in range(4):
                    nc.sync.dma_start(
                        Vin[:, :, bass.ts(n4, 512)],
                        Vd[:, bass.ts(n4, 512)].rearrange("(t p) n -> p t n", p=128),
                    )
                for wt, wd in ((WqT, WqTd), (WkT, WkTd)):
                    nc.sync.dma_start(
                        wt[:], wd[:].rearrange("(t p) n -> p t n", p=128)
                    )
                nc.sync.dma_start(bqk[:], bqkd[:])
                for n4 in range(4):
                    nc.sync.dma_start(
                        Qin[:, :, bass.ts(n4, 512)],
                        Qd[:, bass.ts(n4, 512)].rearrange("(t p) n -> p t n", p=128),
                    )
                for n4 in range(4):
                    nc.sync.dma_start(
                        Kin[:, :, bass.ts(n4, 512)],
                        Kd[:, bass.ts(n4, 512)].rearrange("(t p) n -> p t n", p=128),
                    )
                for j2 in range(8):
                    nc.sync.dma_start(
                        maskT[:, 2 * j2 : 2 * j2 + 2, :],
                        Md[bass.ts(j2, 256), :].rearrange(
                            "(t p) n -> p t n", p=128
                        ),
                    )

                # --- phase 1: projections ---
                with (
                    tc.tile_pool(name="pp", bufs=4, space="PSUM") as pp,
                    tc.tile_pool(name="pv", bufs=2, space="PSUM") as pv,
                ):
                    # vT[s, e] = sum_c V[c, s] * WvT[c, e], per V S-chunk.
                    # Two s-tiles' accumulation chains interleave so
                    # consecutive MMs hit alternating PSUM banks (a single
                    # bank chain serializes at ~290ns/MM on the PE).
                    for s2 in range(8):
                        pss = [
                            pv.tile([128, VW], F32, tag="pv", name=f"pv{t}")
                            for t in range(2)
                        ]
                        for ci in range(8):
                            for t in range(2):
                                nc.tensor.matmul(
                                    pss[t][:],
                                    lhsT=Vin[:, ci, bass.ts(2 * s2 + t, 128)],
                                    rhs=WvT[:, ci, :],
                                    start=(ci == 0),
                                    stop=(ci == 7),
                                )
                        for t in range(2):
                            nc.vector.tensor_add(
                                vT[:, 2 * s2 + t, :], pss[t][:], bvb[:]
                            )
                        if s2 == 0:
                            # trigger the exp_and_others ACT table load
                            # (~2.7us) now, while ACT is idle, instead of at
                            # the first attention exp
                            nc.scalar.activation(escratch[:], bqk[:], EXP)

                    # q/k projections per S-chunk in DMA-arrival order; the
                    # two pairs' chains interleave (alternating PSUM banks)
                    for qk, (dst, wt, src) in enumerate(
                        ((q_sb, WqT, Qin), (k_sb, WkT, Kin))
                    ):
                        for n4 in range(4):
                            pss = [
                                pp.tile([128, 512], F32, tag="pp", name=f"pp{t}")
                                for t in range(2)
                            ]
                            for ci in range(8):
                                for p in range(2):
                                    nc.tensor.matmul(
                                        pss[p][:],
                                        lhsT=wt[:, ci, bass.ts(p, 128)],
                                        rhs=src[:, ci, bass.ts(n4, 512)],
                                        start=(ci == 0),
                                        stop=(ci == 7),
                                    )
                            for p in range(2):
                                nc.scalar.add(
                                    dst[:, p, bass.ts(n4, 512)],
                                    pss[p][:],
                                    bqk[:, 2 * p + qk : 2 * p + qk + 1],
                                )

            # --- phase 2: attention ---
            # (pt/i16/ob reuse Vin's SBUF space; vT proj finished long ago)
            with (
                tc.tile_pool(name="pt", bufs=6) as ptp,
                tc.tile_pool(name="i16", bufs=3) as i16p,
                tc.tile_pool(name="ob", bufs=2) as obp,
                # cx first so it aliases the pp banks (freed by the late proj
                # drains, first written ~2.5us into attention); sc then
                # aliases pv (freed early) + virgin space, so the first score
                # MM issues back-to-back after the last proj MM.
                tc.tile_pool(name="cx", bufs=2, space="PSUM") as cxp,
                tc.tile_pool(name="sc", bufs=2, space="PSUM") as scp,
            ):
                for p in range(2):
                    for hf in range(2):
                        cx = [
                            cxp.tile([65, 2, 512], F32, tag="cx", name=f"cx{i}")
                            for i in range(2)
                        ]

                        def emit_ctx(j, pts):
                            for hh in range(2):
                                hloc = 2 * p + hh
                                for ib in range(2):
                                    nc.tensor.matmul(
                                        cx[hh][:, ib, :],
                                        lhsT=vT[:, j, bass.ds(hloc * HW, 65)],
                                        rhs=pts[hh][:, bass.ts(ib, 512)],
                                        start=(j == 0),
                                        stop=(j == 15),
                                    )

                        # software pipeline, 2-deep: PE emits scores(j) then
                        # ctx(j-2), so ctx's wait on the mask-mul (DVE/GP,
                        # which spikes on Schraudolph j's) never blocks the
                        # next scores -> ACT always has a fresh sc tile
                        pending = []
                        for j in range(16):
                            # both heads' score MMs emitted adjacently so the
                            # PE runs them concurrently (row groups 0-1 / 2-3)
                            sc0 = scp.tile([128, 1024], F32, tag="sc")
                            sc1 = scp.tile([128, 1024], F32, tag="sc")
                            for ib in range(2):
                                for hh, sc in ((0, sc0), (1, sc1)):
                                    lo, hi = 64 * hh, 64 * hh + 64
                                    nc.tensor.matmul(
                                        sc[:, bass.ts(ib, 512)],
                                        lhsT=k_sb[lo:hi, p, bass.ts(j, 128)],
                                        rhs=q_sb[
                                            lo:hi,
                                            p,
                                            bass.ds(hf * 1024 + ib * 512, 512),
                                        ],
                                        start=True,
                                        stop=True,
                                    )
                            if len(pending) >= 2:
                                emit_ctx(*pending.pop(0))
                            pts = []
                            for hh, sc in ((0, sc0), (1, sc1)):
                                pt = ptp.tile([128, 1024], BF, tag="pt")
                                msk = maskT[:, j, bass.ts(hf, 1024)]
                                nc.scalar.activation(
                                    pt[:], sc[:], EXP, scale=ESCALE
                                )
                                nc.vector.tensor_mul(pt[:], pt[:], msk)
                                pts.append(pt)
                            pending.append((j, pts))
                        for item in pending:
                            emit_ctx(*item)
                        # drain: one copy on DVE, one on ACT so the cx banks
                        # release quickly
                        for hh in range(2):
                            hloc = 2 * p + hh
                            ob = obp.tile([65, 2, 512], BF, tag="ob")
                            if hh == 0:
                                nc.vector.tensor_copy(ob[:], cx[hh][:])
                            else:
                                nc.scalar.copy(ob[:], cx[hh][:])
                            nc.sync.dma_start(
                                Od[
                                    bass.ds(hloc * HW, 65), bass.ts(hf, 1024)
                                ].rearrange("p (x y) -> p x y", x=2),
                                ob[:],
                            )
    nc.compile()
    return nc


def _get_nc():
    if "nc" not in _NC_CACHE:
        _NC_CACHE["nc"] = build_nc()
    return _NC_CACHE["nc"]


def _make_in_maps(Q, K, V, mask, Wq, bq, Wk, bk, Wv, bv):
    per_batch = []
    for b in range(B):
        Qa = Q[b].astype(NBF)
        Ka = K[b].astype(NBF)
        Va = V[b].astype(NBF)
        mT = np.ascontiguousarray((~mask[b]).T).astype(np.float32).astype(NBF)
        per_batch.append((Qa, Ka, Va, mT))

    in_maps = []
    for c in range(N_CORES):
        b, g = divmod(c, 4)
        hs = slice(g * CPC, (g + 1) * CPC)
        Qa, Ka, Va, mT = per_batch[b]
        WqTa = np.ascontiguousarray(Wq[hs].T).astype(NBF)
        WkTa = np.ascontiguousarray(Wk[hs].T).astype(NBF)
        WvTa = np.zeros((C, VW), np.float32)
        bvba = np.zeros((128, VW), np.float32)
        for hh in range(HPC):
            ch = slice((g * HPC + hh) * CPH, (g * HPC + hh + 1) * CPH)
            WvTa[:, hh * HW : hh * HW + 64] = Wv[ch].T
            bvba[:, hh * HW : hh * HW + 64] = bv[ch][None, :]
            bvba[:, hh * HW + 64] = 1.0
        # bias for q/k psum->sbuf copies: col 2p+qk = per-partition bias of
        # pair p's 128 channels (rows 0:64 = head 2p, 64:128 = head 2p+1)
        bqka = np.zeros((128, 4), np.float32)
        for p in range(2):
            ch = slice((g * 2 + p) * 128, (g * 2 + p + 1) * 128)
            bqka[:, 2 * p] = bq[ch]
            bqka[:, 2 * p + 1] = bk[ch]
        in_maps.append(
            {
                "Qin": Qa,
                "Kin": Ka,
                "Vin": Va,
                "WqT": WqTa,
                "WkT": WkTa,
                "WvT": WvTa.astype(NBF),
                "bqk": bqka,
                "bvb": bvba,
                "maskT": mT,
            }
        )
    return in_maps


def _assemble(results):
    out = np.zeros((B, S, C), np.float32)
    for c in range(N_CORES):
        b, g = divmod(c, 4)
        o = results[c]["out"].astype(np.float32)  # [260, 2048]
        for hh in range(HPC):
            ctx = o[hh * HW : hh * HW + 64]  # [64, S] = (d, i)
            den = o[hh * HW + 64]  # [S]
            ch0 = (g * HPC + hh) * CPH
            out[b, :, ch0 : ch0 + CPH] = (ctx / den[None, :]).T
    return out


def run(inputs, trace=False):
    in_maps = _make_in_maps(
        np.asarray(inputs["Q"], np.float32),
        np.asarray(inputs["K"], np.float32),
        np.asarray(inputs["V"], np.float32),
        np.asarray(inputs["mask"]),
        np.asarray(inputs["Wq"], np.float32),
        np.asarray(inputs["bq"], np.float32),
        np.asarray(inputs["Wk"], np.float32),
        np.asarray(inputs["bk"], np.float32),
        np.asarray(inputs["Wv"], np.float32),
        np.asarray(inputs["bv"], np.float32),
    )
    br = run_bass_kernel_spmd(_get_nc(), in_maps, list(range(N_CORES)), trace=trace)
    return _assemble(br.results), br


def kernel(**inputs) -> np.ndarray:
    out, _ = run(inputs)
    return out


# revision 20
# speedup vs baseline: 1.0768x; 1.0768x over previous
"""Trainium2 Bass kernel for nn_MultiHeadAttention (B=2, C=1024, H=16, S=2048).

Sharding: 8 cores = 2 batches x 4 head-groups (4 heads per core).

Design notes (vs original baseline):
  - All SBUF allocated up-front except Vin (scoped; its space is reused for
    the attention-phase pt/i16/ob tiles long after vT proj finished), and
    the mask is DMA'd right behind Q/K so the attention phase never stalls
    on SBUF frees.  The baseline's proj->attention PE idle gap tripped the
    HAM clock-gate to K=4/8, which then sticks for tens of 3.4us HAM
    windows (up to ~200us at half clock) -- the single biggest cost.
  - PSUM handoff: in the attention scope, cx is allocated BELOW sc so that
    sc aliases only the early-freed pv banks + virgin space; the first
    score matmul then issues back-to-back after the last proj matmul.
  - A burst of standalone LDWEIGHTS fills the PE pipeline-head bubble at
    the proj->attention transition (PE must wait ~1.2us for the first exp
    to free an sc slot; an idle HAM window there re-throttles the clock).
  - Everything bf16 (fp8 on any operand feeding attention fails the 2e-2
    gate: attention weight concentrates on few keys, so quantization noise
    does not average out).  q_sb/k_sb bf16 (baseline fp32r cost +130ns/MM
    and +140ns/LDW on score matmuls); 1/sqrt(cph) folded into the exp.
  - Q/K DMA'd in 4 S-chunks each, proj emitted per chunk, so the PE rides
    just behind the DMA stream.
  - Scores computed TRANSPOSED (scoresT[j,i] = k.q) so softmax
    normalization rides the ctx matmul via a ones-column in vT.
  - exp on ACT for most tiles; ~1/6 of tiles use a Schraudolph fast-exp on
    DVE (i16 = round(s*A+B) bitcast to bf16) to balance ACT vs DVE load.
    Mask applied multiplicatively after exp (bf16 2x mode on DVE).
  - Output bf16; host divides by the denominator row and transposes.
"""

import numpy as np
import ml_dtypes

import concourse.bass as bass
import concourse.mybir as mybir
import concourse.tile as tile
from concourse import bacc
from concourse.bass_utils import run_bass_kernel_spmd

B = 2
C = 1024
HEADS = 16
CPH = 64
S = 2048
N_CORES = 8
HPC = 4  # heads per core
CPC = HPC * CPH  # channels per core = 256
HW = 65  # per-head slot width in vT (64 d + 1 ones)
VW = HPC * HW  # 260

BF = mybir.dt.bfloat16
F32 = mybir.dt.float32
I16 = mybir.dt.int16
EXP = mybir.ActivationFunctionType.Exp
MULT = mybir.AluOpType.mult
ADD = mybir.AluOpType.add

NBF = ml_dtypes.bfloat16

ESCALE = 1.0 / 8.0  # 1/sqrt(cph), applied inside the exp
# Schraudolph bf16 fast-exp: exp(x*ESCALE) ~= bitcast_bf16(i16(x*SA + SB))
SA = (128.0 / np.log(2.0)) * ESCALE
SB = 128.0 * (127.0 - 0.0577)  # mean-centering constant


def _schraudolph(j, hh):
    return hh == 1 and j % 3 == 2  # 5 of 32 tiles per (p, hf) group


_NC_CACHE = {}


def build_nc():
    nc = bacc.Bacc("TRN2", target_bir_lowering=False)

    Qd = nc.declare_dram_parameter("Qin", [C, S], BF, isOutput=False)
    Kd = nc.declare_dram_parameter("Kin", [C, S], BF, isOutput=False)
    Vd = nc.declare_dram_parameter("Vin", [C, S], BF, isOutput=False)
    WqTd = nc.declare_dram_parameter("WqT", [C, CPC], BF, isOutput=False)
    WkTd = nc.declare_dram_parameter("WkT", [C, CPC], BF, isOutput=False)
    WvTd = nc.declare_dram_parameter("WvT", [C, VW], BF, isOutput=False)
    bqkd = nc.declare_dram_parameter("bqk", [128, 4], F32, isOutput=False)
    bvbd = nc.declare_dram_parameter("bvb", [128, VW], F32, isOutput=False)
    Md = nc.declare_dram_parameter("maskT", [S, S], BF, isOutput=False)
    Od = nc.declare_dram_parameter("out", [VW, S], BF, isOutput=True)

    with tile.TileContext(nc) as tc:
        with (
            tc.tile_pool(name="w", bufs=1) as wp,
            tc.tile_pool(name="io", bufs=1) as io,
            tc.tile_pool(name="qksb", bufs=1) as qkp,
            tc.tile_pool(name="msk", bufs=1) as mkp,
        ):
            # --- persistent SBUF ---
            WvT = wp.tile([128, 8, VW], BF, tag="wv")
            WqT = wp.tile([128, 8, CPC], BF, tag="wq")
            WkT = wp.tile([128, 8, CPC], BF, tag="wk")
            bqk = wp.tile([128, 4], F32, tag="bqk")
            bvb = wp.tile([128, VW], F32, tag="bvb")
            Qin = io.tile([128, 8, S], BF, tag="qi")
            Kin = io.tile([128, 8, S], BF, tag="ki")
            q_sb = qkp.tile([128, 2, S], BF, tag="q")  # pair-major, head rows 0:64/64:128
            k_sb = qkp.tile([128, 2, S], BF, tag="k")
            vT = qkp.tile([128, 16, VW], BF, tag="vt")  # s_tile-major
            maskT = mkp.tile([128, 16, S], BF, tag="m")

            with tc.tile_pool(name="vio", bufs=1) as vio:
                Vin = vio.tile([128, 8, S], BF, tag="vi")

                # --- DMA, in need-order (the shared HW queue drains roughly
                # in posting order at full aggregate bandwidth) ---
                nc.sync.dma_start(
                    WvT[:], WvTd[:].rearrange("(t p) n -> p t n", p=128)
                )
                nc.sync.dma_start(bvb[:], bvbd[:])
                nc.sync.dma_start(Vin[:], Vd[:].rearrange("(t p) n -> p t n", p=128))
                for wt, wd in ((WqT, WqTd), (WkT, WkTd)):
                    nc.sync.dma_start(
                        wt[:], wd[:].rearrange("(t p) n -> p t n", p=128)
                    )
                nc.sync.dma_start(bqk[:], bqkd[:])
                for n4 in range(4):
                    nc.sync.dma_start(
                        Qin[:, :, bass.ts(n4, 512)],
                        Qd[:, bass.ts(n4, 512)].rearrange("(t p) n -> p t n", p=128),
                    )
                for n4 in range(4):
                    nc.sync.dma_start(
                        Kin[:, :, bass.ts(n4, 512)],
                        Kd[:, bass.ts(n4, 512)].rearrange("(t p) n -> p t n", p=128),
                    )
                for j in range(16):
                    nc.sync.dma_start(maskT[:, j, :], Md[bass.ts(j, 128), :])

                # --- phase 1: projections ---
                with (
                    tc.tile_pool(name="pp", bufs=4, space="PSUM") as pp,
                    tc.tile_pool(name="pv", bufs=2, space="PSUM") as pv,
                ):
                    # vT[s, e] = sum_c V[c, s] * WvT[c, e]
                    for s in range(16):
                        ps = pv.tile([128, VW], F32, tag="pv")
                        for ci in range(8):
                            nc.tensor.matmul(
                                ps[:],
                                lhsT=Vin[:, ci, bass.ts(s, 128)],
                                rhs=WvT[:, ci, :],
                                start=(ci == 0),
                                stop=(ci == 7),
                            )
                        nc.vector.tensor_add(vT[:, s, :], ps[:], bvb[:])

                    # q/k projections per S-chunk in DMA-arrival order
                    for qk, (dst, wt, src) in enumerate(
                        ((q_sb, WqT, Qin), (k_sb, WkT, Kin))
                    ):
                        for n4 in range(4):
                            for p in range(2):
                                ps = pp.tile([128, 512], F32, tag="pp")
                                for ci in range(8):
                                    nc.tensor.matmul(
                                        ps[:],
                                        lhsT=wt[:, ci, bass.ts(p, 128)],
                                        rhs=src[:, ci, bass.ts(n4, 512)],
                                        start=(ci == 0),
                                        stop=(ci == 7),
                                    )
                                nc.scalar.add(
                                    dst[:, p, bass.ts(n4, 512)],
                                    ps[:],
                                    bqk[:, 2 * p + qk : 2 * p + qk + 1],
                                )

            # --- phase 2: attention ---
            # (pt/i16/ob reuse Vin's SBUF space; vT proj finished long ago)
            with (
                tc.tile_pool(name="pt", bufs=4) as ptp,
                tc.tile_pool(name="i16", bufs=2) as i16p,
                tc.tile_pool(name="ob", bufs=2) as obp,
                # cx first so it aliases the pp banks (freed by the late proj
                # drains, first written ~2.5us into attention); sc then
                # aliases pv (freed early) + virgin space, so the first score
                # MM issues back-to-back after the last proj MM.
                tc.tile_pool(name="cx", bufs=2, space="PSUM") as cxp,
                tc.tile_pool(name="sc", bufs=2, space="PSUM") as scp,
            ):
                for p in range(2):
                    for hf in range(2):
                        cx = [
                            cxp.tile([65, 2, 512], F32, tag="cx", name=f"cx{i}")
                            for i in range(2)
                        ]
                        for j in range(16):
                            # both heads' score MMs emitted adjacently so the
                            # PE runs them concurrently (row groups 0-1 / 2-3)
                            sc0 = scp.tile([128, 1024], F32, tag="sc")
                            sc1 = scp.tile([128, 1024], F32, tag="sc")
                            for ib in range(2):
                                for hh, sc in ((0, sc0), (1, sc1)):
                                    lo, hi = 64 * hh, 64 * hh + 64
                                    nc.tensor.matmul(
                                        sc[:, bass.ts(ib, 512)],
                                        lhsT=k_sb[lo:hi, p, bass.ts(j, 128)],
                                        rhs=q_sb[
                                            lo:hi,
                                            p,
                                            bass.ds(hf * 1024 + ib * 512, 512),
                                        ],
                                        start=True,
                                        stop=True,
                                    )
                            if p == 0 and hf == 0 and j < 2:
                                # PE filler across the pipeline-head bubble:
                                # the next scores wait ~1.2us for exp(j0) to
                                # free an sc slot, and an idle HAM window
                                # here re-throttles the PE clock for 10s of
                                # microseconds.  Standalone weight loads keep
                                # the PE array busy; the next real matmul
                                # reloads its own weights anyway.
                                for f in range(14):
                                    nc.tensor.ldweights(
                                        k_sb[0:64, p, bass.ts(f % 8, 128)]
                                    )
                            pts = []
                            for hh, sc in ((0, sc0), (1, sc1)):
                                pt = ptp.tile([128, 1024], BF, tag="pt")
                                msk = maskT[:, j, bass.ts(hf, 1024)]
                                if _schraudolph(j, hh):
                                    # fast-exp on DVE: affine+round to i16,
                                    # bitcast to bf16, mask-mul (2x mode)
                                    i16 = i16p.tile([128, 1024], I16, tag="i")
                                    nc.vector.tensor_scalar(
                                        i16[:], sc[:], SA, SB, MULT, ADD
                                    )
                                    nc.vector.tensor_mul(
                                        pt[:], i16[:].bitcast(BF), msk
                                    )
                                else:
                                    nc.scalar.activation(
                                        pt[:], sc[:], EXP, scale=ESCALE
                                    )
                                    nc.vector.tensor_mul(pt[:], pt[:], msk)
                                pts.append(pt)
                            for hh in range(2):
                                hloc = 2 * p + hh
                                for ib in range(2):
                                    nc.tensor.matmul(
                                        cx[hh][:, ib, :],
                                        lhsT=vT[:, j, bass.ds(hloc * HW, 65)],
                                        rhs=pts[hh][:, bass.ts(ib, 512)],
                                        start=(j == 0),
                                        stop=(j == 15),
                                    )
                        # drain: one copy on DVE, one on ACT so the cx banks
                        # release quickly
                        for hh in range(2):
                            hloc = 2 * p + hh
                            ob = obp.tile([65, 2, 512], BF, tag="ob")
                            if hh == 0:
                                nc.vector.tensor_copy(ob[:], cx[hh][:])
                            else:
                                nc.scalar.copy(ob[:], cx[hh][:])
                            nc.sync.dma_start(
                                Od[
                                    bass.ds(hloc * HW, 65), bass.ts(hf, 1024)
                                ].rearrange("p (x y) -> p x y", x=2),
                                ob[:],
                            )
    nc.compile()
    return nc


def _get_nc():
    if "nc" not in _NC_CACHE:
        _NC_CACHE["nc"] = build_nc()
    return _NC_CACHE["nc"]


def _make_in_maps(Q, K, V, mask, Wq, bq, Wk, bk, Wv, bv):
    per_batch = []
    for b in range(B):
        Qa = Q[b].astype(NBF)
        Ka = K[b].astype(NBF)
        Va = V[b].astype(NBF)
        mT = np.ascontiguousarray((~mask[b]).T).astype(np.float32).astype(NBF)
        per_batch.append((Qa, Ka, Va, mT))

    in_maps = []
    for c in range(N_CORES):
        b, g = divmod(c, 4)
        hs = slice(g * CPC, (g + 1) * CPC)
        Qa, Ka, Va, mT = per_batch[b]
        WqTa = np.ascontiguousarray(Wq[hs].T).astype(NBF)
        WkTa = np.ascontiguousarray(Wk[hs].T).astype(NBF)
        WvTa = np.zeros((C, VW), np.float32)
        bvba = np.zeros((128, VW), np.float32)
        for hh in range(HPC):
            ch = slice((g * HPC + hh) * CPH, (g * HPC + hh + 1) * CPH)
            WvTa[:, hh * HW : hh * HW + 64] = Wv[ch].T
            bvba[:, hh * HW : hh * HW + 64] = bv[ch][None, :]
            bvba[:, hh * HW + 64] = 1.0
        # bias for q/k psum->sbuf copies: col 2p+qk = per-partition bias of
        # pair p's 128 channels (rows 0:64 = head 2p, 64:128 = head 2p+1)
        bqka = np.zeros((128, 4), np.float32)
        for p in range(2):
            ch = slice((g * 2 + p) * 128, (g * 2 + p + 1) * 128)
            bqka[:, 2 * p] = bq[ch]
            bqka[:, 2 * p + 1] = bk[ch]
        in_maps.append(
            {
                "Qin": Qa,
                "Kin": Ka,
                "Vin": Va,
                "WqT": WqTa,
                "WkT": WkTa,
                "WvT": WvTa.astype(NBF),
                "bqk": bqka,
                "bvb": bvba,
                "maskT": mT,
            }
        )
    return in_maps


def _assemble(results):
    out = np.zeros((B, S, C), np.float32)
    for c in range(N_CORES):
        b, g = divmod(c, 4)
        o = results[c]["out"].astype(np.float32)  # [260, 2048]
        for hh in range(HPC):
            ctx = o[hh * HW : hh * HW + 64]  # [64, S] = (d, i)
            den = o[hh * HW + 64]  # [S]
            ch0 = (g * HPC + hh) * CPH
            out[b, :, ch0 : ch0 + CPH] = (ctx / den[None, :]).T
    return out


def run(inputs, trace=False):
    in_maps = _make_in_maps(
        np.asarray(inputs["Q"], np.float32),
        np.asarray(inputs["K"], np.float32),
        np.asarray(inputs["V"], np.float32),
        np.asarray(inputs["mask"]),
        np.asarray(inputs["Wq"], np.float32),
        np.asarray(inputs["bq"], np.float32),
        np.asarray(inputs["Wk"], np.float32),
        np.asarray(inputs["bk"], np.float32),
        np.asarray(inputs["Wv"], np.float32),
        np.asarray(inputs["bv"], np.float32),
    )
    br = run_bass_kernel_spmd(_get_nc(), in_maps, list(range(N_CORES)), trace=trace)
    return _assemble(br.results), br


def kernel(**inputs) -> np.ndarray:
    out, _ = run(inputs)
    return out


# revision 21
# speedup vs baseline: 1.1028x; 1.0241x over previous
"""Trainium2 Bass kernel for nn_MultiHeadAttention (B=2, C=1024, H=16, S=2048).

Sharding: 8 cores = 2 batches x 4 head-groups (4 heads per core).

Design notes (vs original baseline):
  - All SBUF allocated up-front except Vin (scoped; its space is reused for
    the attention-phase pt/i16/ob tiles long after vT proj finished), and
    the mask is DMA'd right behind Q/K so the attention phase never stalls
    on SBUF frees.  The baseline's proj->attention PE idle gap tripped the
    HAM clock-gate to K=4/8, which then sticks for tens of 3.4us HAM
    windows (up to ~200us at half clock) -- the single biggest cost.
  - PSUM handoff: in the attention scope, cx is allocated BELOW sc so that
    sc aliases only the early-freed pv banks + virgin space; the first
    score matmul then issues back-to-back after the last proj matmul.
  - A burst of standalone LDWEIGHTS fills the PE pipeline-head bubble at
    the proj->attention transition (PE must wait ~1.2us for the first exp
    to free an sc slot; an idle HAM window there re-throttles the clock).
  - Everything bf16 (fp8 on any operand feeding attention fails the 2e-2
    gate: attention weight concentrates on few keys, so quantization noise
    does not average out).  q_sb/k_sb bf16 (baseline fp32r cost +130ns/MM
    and +140ns/LDW on score matmuls); 1/sqrt(cph) folded into the exp.
  - Q/K DMA'd in 4 S-chunks each, proj emitted per chunk, so the PE rides
    just behind the DMA stream.
  - Scores computed TRANSPOSED (scoresT[j,i] = k.q) so softmax
    normalization rides the ctx matmul via a ones-column in vT.
  - exp on ACT for most tiles; ~1/6 of tiles use a Schraudolph fast-exp on
    DVE (i16 = round(s*A+B) bitcast to bf16) to balance ACT vs DVE load.
    Mask applied multiplicatively after exp (bf16 2x mode on DVE).
  - Output bf16; host divides by the denominator row and transposes.
"""

import numpy as np
import ml_dtypes

import concourse.bass as bass
import concourse.mybir as mybir
import concourse.tile as tile
from concourse import bacc
from concourse.bass_utils import run_bass_kernel_spmd

B = 2
C = 1024
HEADS = 16
CPH = 64
S = 2048
N_CORES = 8
HPC = 4  # heads per core
CPC = HPC * CPH  # channels per core = 256
HW = 65  # per-head slot width in vT (64 d + 1 ones)
VW = HPC * HW  # 260

BF = mybir.dt.bfloat16
F32 = mybir.dt.float32
I16 = mybir.dt.int16
EXP = mybir.ActivationFunctionType.Exp
MULT = mybir.AluOpType.mult
ADD = mybir.AluOpType.add

NBF = ml_dtypes.bfloat16

ESCALE = 1.0 / 8.0  # 1/sqrt(cph), applied inside the exp
# Schraudolph bf16 fast-exp: exp(x*ESCALE) ~= bitcast_bf16(i16(x*SA + SB))
SA = (128.0 / np.log(2.0)) * ESCALE
SB = 128.0 * (127.0 - 0.0577)  # mean-centering constant


def _schraudolph(j, hh):
    return hh == 1 and j % 3 == 2  # 5 of 32 tiles per (p, hf) group


_NC_CACHE = {}


def build_nc():
    nc = bacc.Bacc("TRN2", target_bir_lowering=False)

    Qd = nc.declare_dram_parameter("Qin", [C, S], BF, isOutput=False)
    Kd = nc.declare_dram_parameter("Kin", [C, S], BF, isOutput=False)
    Vd = nc.declare_dram_parameter("Vin", [C, S], BF, isOutput=False)
    WqTd = nc.declare_dram_parameter("WqT", [C, CPC], BF, isOutput=False)
    WkTd = nc.declare_dram_parameter("WkT", [C, CPC], BF, isOutput=False)
    WvTd = nc.declare_dram_parameter("WvT", [C, VW], BF, isOutput=False)
    bqkd = nc.declare_dram_parameter("bqk", [128, 4], F32, isOutput=False)
    bvbd = nc.declare_dram_parameter("bvb", [128, VW], F32, isOutput=False)
    Md = nc.declare_dram_parameter("maskT", [S, S], BF, isOutput=False)
    Od = nc.declare_dram_parameter("out", [VW, S], BF, isOutput=True)

    with tile.TileContext(nc) as tc:
        with (
            tc.tile_pool(name="w", bufs=1) as wp,
            tc.tile_pool(name="io", bufs=1) as io,
            tc.tile_pool(name="qksb", bufs=1) as qkp,
            tc.tile_pool(name="msk", bufs=1) as mkp,
        ):
            # --- persistent SBUF ---
            WvT = wp.tile([128, 8, VW], BF, tag="wv")
            WqT = wp.tile([128, 8, CPC], BF, tag="wq")
            WkT = wp.tile([128, 8, CPC], BF, tag="wk")
            bqk = wp.tile([128, 4], F32, tag="bqk")
            bvb = wp.tile([128, VW], F32, tag="bvb")
            Qin = io.tile([128, 8, S], BF, tag="qi")
            Kin = io.tile([128, 8, S], BF, tag="ki")
            q_sb = qkp.tile([128, 2, S], BF, tag="q")  # pair-major, head rows 0:64/64:128
            k_sb = qkp.tile([128, 2, S], BF, tag="k")
            vT = qkp.tile([128, 16, VW], BF, tag="vt")  # s_tile-major
            maskT = mkp.tile([128, 16, S], BF, tag="m")

            with tc.tile_pool(name="vio", bufs=1) as vio:
                Vin = vio.tile([128, 8, S], BF, tag="vi")

                # --- DMA, in need-order (the shared HW queue drains roughly
                # in posting order at full aggregate bandwidth) ---
                nc.sync.dma_start(
                    WvT[:], WvTd[:].rearrange("(t p) n -> p t n", p=128)
                )
                nc.sync.dma_start(bvb[:], bvbd[:])
                nc.sync.dma_start(Vin[:], Vd[:].rearrange("(t p) n -> p t n", p=128))
                for wt, wd in ((WqT, WqTd), (WkT, WkTd)):
                    nc.sync.dma_start(
                        wt[:], wd[:].rearrange("(t p) n -> p t n", p=128)
                    )
                nc.sync.dma_start(bqk[:], bqkd[:])
                for n4 in range(4):
                    nc.sync.dma_start(
                        Qin[:, :, bass.ts(n4, 512)],
                        Qd[:, bass.ts(n4, 512)].rearrange("(t p) n -> p t n", p=128),
                    )
                for n4 in range(4):
                    nc.sync.dma_start(
                        Kin[:, :, bass.ts(n4, 512)],
                        Kd[:, bass.ts(n4, 512)].rearrange("(t p) n -> p t n", p=128),
                    )
                for j in range(16):
                    nc.sync.dma_start(maskT[:, j, :], Md[bass.ts(j, 128), :])

                # --- phase 1: projections ---
                with (
                    tc.tile_pool(name="pp", bufs=4, space="PSUM") as pp,
                    tc.tile_pool(name="pv", bufs=2, space="PSUM") as pv,
                ):
                    # vT[s, e] = sum_c V[c, s] * WvT[c, e]
                    for s in range(16):
                        ps = pv.tile([128, VW], F32, tag="pv")
                        for ci in range(8):
                            nc.tensor.matmul(
                                ps[:],
                                lhsT=Vin[:, ci, bass.ts(s, 128)],
                                rhs=WvT[:, ci, :],
                                start=(ci == 0),
                                stop=(ci == 7),
                            )
                        nc.vector.tensor_add(vT[:, s, :], ps[:], bvb[:])

                    # q/k projections per S-chunk in DMA-arrival order
                    for qk, (dst, wt, src) in enumerate(
                        ((q_sb, WqT, Qin), (k_sb, WkT, Kin))
                    ):
                        for n4 in range(4):
                            for p in range(2):
                                ps = pp.tile([128, 512], F32, tag="pp")
                                for ci in range(8):
                                    nc.tensor.matmul(
                                        ps[:],
                                        lhsT=wt[:, ci, bass.ts(p, 128)],
                                        rhs=src[:, ci, bass.ts(n4, 512)],
                                        start=(ci == 0),
                                        stop=(ci == 7),
                                    )
                                nc.scalar.add(
                                    dst[:, p, bass.ts(n4, 512)],
                                    ps[:],
                                    bqk[:, 2 * p + qk : 2 * p + qk + 1],
                                )

            # --- phase 2: attention ---
            # (pt/i16/ob reuse Vin's SBUF space; vT proj finished long ago)
            with (
                tc.tile_pool(name="pt", bufs=4) as ptp,
                tc.tile_pool(name="i16", bufs=2) as i16p,
                tc.tile_pool(name="ob", bufs=2) as obp,
                # cx first so it aliases the pp banks (freed by the late proj
                # drains, first written ~2.5us into attention); sc then
                # aliases pv (freed early) + virgin space, so the first score
                # MM issues back-to-back after the last proj MM.
                tc.tile_pool(name="cx", bufs=2, space="PSUM") as cxp,
                tc.tile_pool(name="sc", bufs=2, space="PSUM") as scp,
            ):
                for p in range(2):
                    for hf in range(2):
                        cx = [
                            cxp.tile([65, 2, 512], F32, tag="cx", name=f"cx{i}")
                            for i in range(2)
                        ]
                        for j in range(16):
                            # both heads' score MMs emitted adjacently so the
                            # PE runs them concurrently (row groups 0-1 / 2-3)
                            sc0 = scp.tile([128, 1024], F32, tag="sc")
                            sc1 = scp.tile([128, 1024], F32, tag="sc")
                            for ib in range(2):
                                for hh, sc in ((0, sc0), (1, sc1)):
                                    lo, hi = 64 * hh, 64 * hh + 64
                                    nc.tensor.matmul(
                                        sc[:, bass.ts(ib, 512)],
                                        lhsT=k_sb[lo:hi, p, bass.ts(j, 128)],
                                        rhs=q_sb[
                                            lo:hi,
                                            p,
                                            bass.ds(hf * 1024 + ib * 512, 512),
                                        ],
                                        start=True,
                                        stop=True,
                                    )
                            if p == 0 and hf == 0 and j < 2:
                                # PE filler across the pipeline-head bubble:
                                # the next scores wait ~1.2us for exp(j0) to
                                # free an sc slot, and an idle HAM window
                                # here re-throttles the PE clock for 10s of
                                # microseconds.  Standalone weight loads keep
                                # the PE array busy; the next real matmul
                                # reloads its own weights anyway.
                                for f in range(14):
                                    nc.tensor.ldweights(
                                        k_sb[0:64, p, bass.ts(f % 8, 128)]
                                    )
                            pts = []
                            for hh, sc in ((0, sc0), (1, sc1)):
                                pt = ptp.tile([128, 1024], BF, tag="pt")
                                msk = maskT[:, j, bass.ts(hf, 1024)]
                                nc.scalar.activation(
                                    pt[:], sc[:], EXP, scale=ESCALE
                                )
                                nc.vector.tensor_mul(pt[:], pt[:], msk)
                                pts.append(pt)
                            for hh in range(2):
                                hloc = 2 * p + hh
                                for ib in range(2):
                                    nc.tensor.matmul(
                                        cx[hh][:, ib, :],
                                        lhsT=vT[:, j, bass.ds(hloc * HW, 65)],
                                        rhs=pts[hh][:, bass.ts(ib, 512)],
                                        start=(j == 0),
                                        stop=(j == 15),
                                    )
                        # drain: one copy on DVE, one on ACT so the cx banks
                        # release quickly
                        for hh in range(2):
                            hloc = 2 * p + hh
                            ob = obp.tile([65, 2, 512], BF, tag="ob")
                            if hh == 0:
                                nc.vector.tensor_copy(ob[:], cx[hh][:])
                            else:
                                nc.scalar.copy(ob[:], cx[hh][:])
                            nc.sync.dma_start(
                                Od[
                                    bass.ds(hloc * HW, 65), bass.ts(hf, 1024)
                                ].rearrange("p (x y) -> p x y", x=2),
                                ob[:],
                            )
    nc.compile()
    return nc


def _get_nc():
    if "nc" not in _NC_CACHE:
        _NC_CACHE["nc"] = build_nc()
    return _NC_CACHE["nc"]


def _make_in_maps(Q, K, V, mask, Wq, bq, Wk, bk, Wv, bv):
    per_batch = []
    for b in range(B):
        Qa = Q[b].astype(NBF)
        Ka = K[b].astype(NBF)
        Va = V[b].astype(NBF)
        mT = np.ascontiguousarray((~mask[b]).T).astype(np.float32).astype(NBF)
        per_batch.append((Qa, Ka, Va, mT))

    in_maps = []
    for c in range(N_CORES):
        b, g = divmod(c, 4)
        hs = slice(g * CPC, (g + 1) * CPC)
        Qa, Ka, Va, mT = per_batch[b]
        WqTa = np.ascontiguousarray(Wq[hs].T).astype(NBF)
        WkTa = np.ascontiguousarray(Wk[hs].T).astype(NBF)
        WvTa = np.zeros((C, VW), np.float32)
        bvba = np.zeros((128, VW), np.float32)
        for hh in range(HPC):
            ch = slice((g * HPC + hh) * CPH, (g * HPC + hh + 1) * CPH)
            WvTa[:, hh * HW : hh * HW + 64] = Wv[ch].T
            bvba[:, hh * HW : hh * HW + 64] = bv[ch][None, :]
            bvba[:, hh * HW + 64] = 1.0
        # bias for q/k psum->sbuf copies: col 2p+qk = per-partition bias of
        # pair p's 128 channels (rows 0:64 = head 2p, 64:128 = head 2p+1)
        bqka = np.zeros((128, 4), np.float32)
        for p in range(2):
            ch = slice((g * 2 + p) * 128, (g * 2 + p + 1) * 128)
            bqka[:, 2 * p] = bq[ch]
            bqka[:, 2 * p + 1] = bk[ch]
        in_maps.append(
            {
                "Qin": Qa,
                "Kin": Ka,
                "Vin": Va,
                "WqT": WqTa,
                "WkT": WkTa,
                "WvT": WvTa.astype(NBF),
                "bqk": bqka,
                "bvb": bvba,
                "maskT": mT,
            }
        )
    return in_maps


def _assemble(results):
    out = np.zeros((B, S, C), np.float32)
    for c in range(N_CORES):
        b, g = divmod(c, 4)
        o = results[c]["out"].astype(np.float32)  # [260, 2048]
        for hh in range(HPC):
            ctx = o[hh * HW : hh * HW + 64]  # [64, S] = (d, i)
            den = o[hh * HW + 64]  # [S]
            ch0 = (g * HPC + hh) * CPH
            out[b, :, ch0 : ch0 + CPH] = (ctx / den[None, :]).T
    return out


def run(inputs, trace=False):
    in_maps = _make_in_maps(
        np.asarray(inputs["Q"], np.float32),
        np.asarray(inputs["K"], np.float32),
        np.asarray(inputs["V"], np.float32),
        np.asarray(inputs["mask"]),
        np.asarray(inputs["Wq"], np.float32),
        np.asarray(inputs["bq"], np.float32),
        np.asarray(inputs["Wk"], np.float32),
        np.asarray(inputs["bk"], np.float32),
        np.asarray(inputs["Wv"], np.float32),
        np.asarray(inputs["bv"], np.float32),
    )
    br = run_bass_kernel_spmd(_get_nc(), in_maps, list(range(N_CORES)), trace=trace)
    return _assemble(br.results), br


def kernel(**inputs) -> np.ndarray:
    out, _ = run(inputs)
    return out
